# revision 27
# baseline (speedup 1.0000x reference)
"""Causal multi-head attention (B=4, S=2048, D=1024, H=16, HD=64) on 8 NeuronCores.

Sharding: core c handles batch b=c//2 and head-group hg=c%2 (8 heads each).
Each core computes out^T_partial = Wo_hg^T @ ctx_hg^T for its (b, hg); the host
sums the two head-group partials per batch, transposes, and adds the bias.

v2 structure (superblock-major, packed scores):
- Heads are processed in pairs (2p, 2p+1) living at SBUF partitions 0-63 /
  64-127 of qT/kT tile m=p.  The two heads' score matmuls (contraction K=64)
  are issued back-to-back at tile positions (0,0)/(64,0) so they run
  concurrently in disjoint row-groups of the PE array.
- Attention runs superblock-major (all pairs finish q-superblock I before
  I+1), with QKV-projection / out-projection matmuls statically interleaved
  between exp-dependent attention matmuls so the PE never waits on ScalarE.
- Softmax denominators come from a ones-column appended to V; normalization
  uses a reciprocal at partition 64 plus a K=1 broadcast matmul (no DMA).
- Output partials are written in bf16; the host sums in f32 and adds bias.
"""

import sys

for _p in ("/opt/trn_rl_repo",):
    if _p not in sys.path:
        sys.path.insert(0, _p)

import numpy as np
import ml_dtypes
from contextlib import ExitStack

import concourse.bacc as bacc
import concourse.tile as tile
from concourse import mybir
from concourse.bass_utils import run_bass_kernel_spmd

F32 = mybir.dt.float32
BF16 = mybir.dt.bfloat16
Exp = mybir.ActivationFunctionType.Exp
Mult = mybir.AluOpType.mult

B, S, D, H, HD = 4, 2048, 1024, 16, 64
NC = 8          # cores
HL = 8          # heads per core (head-group)
DH = HL * HD    # 512, per-core head dim
KT = D // 128   # 8 k-tiles over d_in
ST = S // 128   # 16 tiles over sequence
NB = S // 512   # 4 q-superblocks
NP = HL // 2    # 4 head pairs per core
SCALE = 1.0 / np.sqrt(HD)


def _build_nc(debug=False):
    nc = bacc.Bacc("TRN2", target_bir_lowering=False)

    xT = nc.declare_dram_parameter("xT", [D, S], BF16, isOutput=False)
    wq = nc.declare_dram_parameter("wq", [D, DH], BF16, isOutput=False)
    wk = nc.declare_dram_parameter("wk", [D, DH], BF16, isOutput=False)
    wv = nc.declare_dram_parameter("wv", [D, DH], BF16, isOutput=False)
    wo = nc.declare_dram_parameter("wo", [DH, D], BF16, isOutput=False)
    tri = nc.declare_dram_parameter("tri", [128, 128], BF16, isOutput=False)
    outT = nc.declare_dram_parameter("outT", [D, S], BF16, isOutput=True)
    if debug:
        d_qT = nc.declare_dram_parameter("d_qT", [DH, S], BF16, isOutput=True)
        d_kT = nc.declare_dram_parameter("d_kT", [DH, S], BF16, isOutput=True)
        d_v = nc.declare_dram_parameter("d_v", [S, HL * (HD + 1)], BF16, isOutput=True)
        d_ctxT = nc.declare_dram_parameter("d_ctxT", [DH, S], BF16, isOutput=True)

    with tile.TileContext(nc) as tc, ExitStack() as ctx:
        const_pool = ctx.enter_context(tc.tile_pool(name="const", bufs=1))
        x_pool = ctx.enter_context(tc.tile_pool(name="x", bufs=1))
        w_pool = ctx.enter_context(tc.tile_pool(name="w", bufs=1))
        qk_pool = ctx.enter_context(tc.tile_pool(name="qk", bufs=1))
        v_pool = ctx.enter_context(tc.tile_pool(name="v", bufs=1))
        ctxT_pool = ctx.enter_context(tc.tile_pool(name="ctxT", bufs=1))
        e_pool = ctx.enter_context(tc.tile_pool(name="e", bufs=4))
        r_pool = ctx.enter_context(tc.tile_pool(name="r", bufs=1))
        o_pool = ctx.enter_context(tc.tile_pool(name="o", bufs=2))
        o3_pool = ctx.enter_context(tc.tile_pool(name="o3", bufs=1))
        ps_sp = ctx.enter_context(tc.tile_pool(name="ps_sp", bufs=2, space="PSUM"))
        ps_c = ctx.enter_context(tc.tile_pool(name="ps_c", bufs=1, space="PSUM"))
        ps_gen = ctx.enter_context(tc.tile_pool(name="ps_gen", bufs=2, space="PSUM"))

        # ---- constants ----
        trit = const_pool.tile([128, 128], BF16)
        nc.sync.dma_start(trit[:], tri[:])
        onesb = const_pool.tile([1, 64], BF16)
        nc.vector.memset(onesb[:], 1.0)

        # ---- inputs ----
        # Per-k 2D slices (multi-dim dram APs under-synchronize on HW).
        # Triggers are spread across engine queues so the front-critical
        # loads (wv + x superblock 0) issue in parallel.
        wvt = w_pool.tile([128, KT, DH], BF16, name="wvt")
        wqt = w_pool.tile([128, KT, DH], BF16, name="wqt")
        wkt = w_pool.tile([128, KT, DH], BF16, name="wkt")
        wot = w_pool.tile([128, DH // 128, D], BF16, name="wot")
        xs = [x_pool.tile([128, KT, 512], BF16, name=f"xs{_}") for _ in range(NB)]

        # Priority order on one queue: a DMA's descriptors go out before the
        # next trigger's, so the front-critical tensors finish first instead
        # of fair-sharing HBM bandwidth with the whole load set.
        def xsrc(s):
            return xT[:, 512 * s : 512 * (s + 1)].rearrange("(k p) c -> p k c", p=128)

        for k in range(KT):
            nc.sync.dma_start(wvt[:, k, :], wv[128 * k : 128 * (k + 1), :])
            nc.sync.dma_start(xs[0][:, k, :], xT[128 * k : 128 * (k + 1), 0:512])
        nc.sync.dma_start(wqt[:], wq.rearrange("(k p) c -> p k c", p=128)[:])
        nc.sync.dma_start(wkt[:], wk.rearrange("(k p) c -> p k c", p=128)[:])
        nc.sync.dma_start(xs[1][:], xsrc(1)[:])
        nc.sync.dma_start(xs[2][:], xsrc(2)[:])
        nc.sync.dma_start(xs[3][:], xsrc(3)[:])
        nc.sync.dma_start(wot[:], wo.rearrange("(k p) c -> p k c", p=128)[:])

        # ---- persistent activations ----
        qTt = [qk_pool.tile([128, S], BF16, name=f"qT{_}") for _ in range(NP)]
        kTt = [qk_pool.tile([128, S], BF16, name=f"kT{_}") for _ in range(NP)]
        vt = [v_pool.tile([128, HL * (HD + 1)], BF16, name=f"v{_}") for _ in range(ST)]
        ctxT = [ctxT_pool.tile([128, S], BF16, name=f"ctxT{_}") for _ in range(NP)]

        # ---- generation units ----
        # Each unit is one PSUM accumulation chain, split into two half-unit
        # closures (~4 matmuls each) for fine-grained interleaving with the
        # attention rounds.  Interleaving foreign matmuls inside an open
        # accumulation group is legal (per-bank has_written state).
        def emit_v(st):
            """V for k-tile st: [128, 8*(HD+1)] with a ones column per head."""
            state = {}
            xcol = xs[st // 4]

            def half_a():
                nc.vector.memset(
                    vt[st].rearrange("p (h c) -> p h c", c=HD + 1)[:, :, HD], 1.0
                )
                state["pv"] = ps_gen.tile([128, 512], F32, tag="pg", name=f"pv{st}")
                for k in range(KT // 2):
                    nc.tensor.matmul(
                        state["pv"][:],
                        xcol[:, k, 128 * (st % 4) : 128 * (st % 4 + 1)],
                        wvt[:, k, :],
                        start=(k == 0),
                        stop=False,
                    )

            def half_b():
                pv = state["pv"]
                for k in range(KT // 2, KT):
                    nc.tensor.matmul(
                        pv[:],
                        xcol[:, k, 128 * (st % 4) : 128 * (st % 4 + 1)],
                        wvt[:, k, :],
                        start=False,
                        stop=(k == KT - 1),
                    )
                nc.vector.tensor_copy(
                    vt[st].rearrange("p (h c) -> p h c", c=HD + 1)[:, :, 0:HD],
                    pv.rearrange("p (h c) -> p h c", c=HD)[:],
                )

            return [half_a, half_b]

        def emit_qk(wt, dst, m, n):
            """q^T or k^T for head-pair m, sequence superblock n."""
            state = {}

            def half_a():
                state["ps"] = ps_gen.tile([128, 512], F32, tag="pg", name=f"pqk{m}_{n}")
                for k in range(KT // 2):
                    nc.tensor.matmul(
                        state["ps"][:],
                        wt[:, k, 128 * m : 128 * (m + 1)],
                        xs[n][:, k, :],
                        start=(k == 0),
                        stop=False,
                    )

            def half_b():
                ps = state["ps"]
                for k in range(KT // 2, KT):
                    nc.tensor.matmul(
                        ps[:],
                        wt[:, k, 128 * m : 128 * (m + 1)],
                        xs[n][:, k, :],
                        start=False,
                        stop=(k == KT - 1),
                    )
                nc.vector.tensor_copy(dst[m][:, 512 * n : 512 * (n + 1)], ps[:])

            return [half_a, half_b]

        def emit_op(m, n):
            """out^T rows [128m:128(m+1)], columns superblock n."""

            def unit():
                ps = ps_gen.tile([128, 512], F32, tag="pg", name=f"pop{m}_{n}")
                for k in range(DH // 128):
                    nc.tensor.matmul(
                        ps[:],
                        wot[:, k, 128 * m : 128 * (m + 1)],
                        ctxT[k][:, 512 * n : 512 * (n + 1)],
                        start=(k == 0),
                        stop=(k == DH // 128 - 1),
                    )
                ot = o_pool.tile([128, 512], BF16, tag="ot", name=f"ot{m}_{n}")
                nc.vector.tensor_copy(ot[:], ps[:])
                nc.sync.dma_start(
                    outT[128 * m : 128 * (m + 1), 512 * n : 512 * (n + 1)], ot[:]
                )

            return [unit]

        # Final column-superblock out-proj, split so only a rank-128 update
        # plus an add remains after the last head finishes.
        o3_tiles = {}

        def emit_op3_partial(m):
            def unit():
                ps = ps_gen.tile([128, 512], F32, tag="pg", name=f"pop3a{m}")
                for k in range(3):
                    nc.tensor.matmul(
                        ps[:],
                        wot[:, k, 128 * m : 128 * (m + 1)],
                        ctxT[k][:, 1536:2048],
                        start=(k == 0),
                        stop=(k == 2),
                    )
                t = o3_pool.tile([128, 512], F32, tag=f"o3_{m}", name=f"o3_{m}")
                nc.vector.tensor_copy(t[:], ps[:])
                o3_tiles[m] = t

            return [unit]

        def emit_op3_final(m):
            def unit():
                ps = ps_gen.tile([128, 512], F32, tag="pg", name=f"pop3b{m}")
                nc.tensor.matmul(
                    ps[:],
                    wot[:, 3, 128 * m : 128 * (m + 1)],
                    ctxT[3][:, 1536:2048],
                    start=True,
                    stop=True,
                )
                ot = o_pool.tile([128, 512], BF16, tag="ot", name=f"ot3_{m}")
                nc.vector.tensor_tensor(
                    ot[:], o3_tiles[m][:], ps[:], mybir.AluOpType.add
                )
                nc.sync.dma_start(outT[128 * m : 128 * (m + 1), 1536:2048], ot[:])

            return [unit]

        # ---- attention ----
        pending = []  # deferred normalization closures

        def make_norm(p, I, X, cps):
            def _norm():
                cun = r_pool.tile([65, 512], F32, tag=f"cun{X}", name="cun")
                nc.vector.tensor_copy(cun[:], cps[:])
                # den row must move to partition 0: reciprocal_approx_fast is
                # custom DVE ucode and corrupts SBUF at a nonzero base
                # partition (HW-only; CoreSim doesn't model it).
                den0 = r_pool.tile([1, 512], F32, tag="den0", name="den0")
                nc.sync.dma_start(den0[0:1, :], cun[64:65, :])
                rec = r_pool.tile([1, 512], F32, tag="rec", name="rec")
                nc.vector.reciprocal_approx_fast(rec[0:1, :], den0[0:1, :])
                recb = r_pool.tile([1, 512], BF16, tag="recb", name="recb")
                nc.vector.tensor_copy(recb[0:1, :], rec[0:1, :])
                bc = ps_c.tile([65, 512], F32, tag=f"c{X}", name="bc")
                nc.tensor.matmul(
                    bc[0:64, :], onesb[0:1, 0:64], recb[0:1, :],
                    start=True, stop=True,
                )
                dst = ctxT[p][64 * X : 64 * X + 64, 512 * I : 512 * (I + 1)]
                if X == 0:
                    nc.vector.tensor_tensor(dst, cun[0:64, :], bc[0:64, :], Mult)
                else:
                    nrm = r_pool.tile([64, 512], BF16, tag="nrm", name="nrm")
                    nc.vector.tensor_tensor(nrm[:], cun[0:64, :], bc[0:64, :], Mult)
                    nc.sync.dma_start(dst, nrm[:])

            return _norm

        def attn_pair(p, I, fillers, budget, urgent=None):
            """All k-rounds for head pair p, q-superblock I.

            fillers: deque of half-unit closures; budget[0] accumulates the
            fill pacing fraction per round.  urgent: closures popped one per
            round ahead of the budgeted fillers (deadline-critical units)."""
            nj = 4 * I + 4
            cps = [
                ps_c.tile([65, 512], F32, tag=f"c{X}", name=f"cps{X}")
                for X in range(2)
            ]
            for j in range(nj):
                diag = (j // 4 == I)
                lo = 128 * (j - 4 * I) if diag else 0
                sp = ps_sp.tile([128, 1024], F32, tag="sp", name="sp")
                if lo > 0:
                    # B-head's masked hole would be uninitialized PSUM under
                    # the single merged exp below.
                    nc.vector.memset(sp[:, 512 : 512 + lo], 0.0)
                for X in range(2):
                    nc.tensor.matmul(
                        sp[:, 512 * X + lo : 512 * (X + 1)],
                        kTt[p][64 * X : 64 * X + 64, 128 * j : 128 * (j + 1)],
                        qTt[p][64 * X : 64 * X + 64, 512 * I + lo : 512 * (I + 1)],
                        start=True,
                        stop=True,
                    )
                e = e_pool.tile([128, 1024], BF16, tag="e", name="e")
                nc.scalar.activation(
                    e[:, lo:1024], sp[:, lo:1024], Exp, scale=float(SCALE)
                )
                if diag:
                    for X in range(2):
                        nc.vector.tensor_tensor(
                            e[:, 512 * X + lo : 512 * X + lo + 128],
                            e[:, 512 * X + lo : 512 * X + lo + 128],
                            trit[:],
                            Mult,
                        )
                while pending:
                    pending.pop(0)()
                if urgent:
                    urgent.pop(0)()
                budget[0] += budget[1]
                while budget[0] >= 1.0 and fillers:
                    fillers.pop(0)()
                    budget[0] -= 1.0
                for X in range(2):
                    nc.tensor.matmul(
                        cps[X][:, lo:512],
                        vt[j][:, (HD + 1) * (2 * p + X) : (HD + 1) * (2 * p + X + 1)],
                        e[:, 512 * X + lo : 512 * (X + 1)],
                        start=(j == 0),
                        stop=(j == nj - 1),
                        skip_group_check=True,
                    )
            for X in range(2):
                pending.append(make_norm(p, I, X, cps[X]))

        # ---- emission schedule ----
        def run_all(units):
            for u in units:
                for half in u:
                    half()

        def flat(units):
            return [half for u in units for half in u]

        # upfront: V k-tiles 0-3 and q/k for pair 0, superblock 0
        run_all([emit_v(st) for st in range(4)])
        run_all([emit_qk(wqt, qTt, 0, 0), emit_qk(wkt, kTt, 0, 0)])

        phase_fillers = [
            # during sb0: remaining sb0 q/k, V 4-7, all of sb1 q/k
            flat(
                [emit_qk(wqt, qTt, m, 0) for m in range(1, NP)]
                + [emit_qk(wkt, kTt, m, 0) for m in range(1, NP)]
                + [emit_v(st) for st in range(4, 8)]
                + [emit_qk(wqt, qTt, m, 1) for m in range(NP)]
                + [emit_qk(wkt, kTt, m, 1) for m in range(NP)]
            ),
            # during sb1: sb2 q/k, V 8-11
            flat(
                [emit_qk(wqt, qTt, m, 2) for m in range(NP)]
                + [emit_qk(wkt, kTt, m, 2) for m in range(NP)]
                + [emit_v(st) for st in range(8, 12)]
            ),
            # during sb2: sb3 q/k, out-proj columns sb0
            flat(
                [emit_qk(wqt, qTt, m, 3) for m in range(NP)]
                + [emit_qk(wkt, kTt, m, 3) for m in range(NP)]
                + [emit_op(m, 0) for m in range(D // 128)]
            ),
            # during sb3: out-proj columns sb1+sb2 (V 12-15 go in the
            # urgent lane: needed by round 12 = pair 0's j=12)
            flat(
                [emit_op(m, 1) for m in range(D // 128)]
                + [emit_op(m, 2) for m in range(D // 128)]
            ),
        ]
        # urgent lanes, popped one per round ahead of budgeted fillers:
        #  (I=3, p=0): V 12-15 (needed from pair 0's j=12 round on)
        #  (I=3, p=3): final out-proj partials (k-chunks 0-2 are complete
        #              once pair 2's norms land at pair 3's round 0)
        urgent_lanes = {
            (3, 0): flat([emit_v(st) for st in range(12, 16)]),
            (3, 3): flat([emit_op3_partial(m) for m in range(D // 128)]),
        }

        for I in range(NB):
            fillers = phase_fillers[I]
            rounds = NP * (4 * I + 4)
            budget = [0.999, len(fillers) / rounds]
            for p in range(NP):
                urgent = urgent_lanes.get((I, p), [])
                attn_pair(p, I, fillers, budget, urgent)
                while urgent:
                    urgent.pop(0)()
            while fillers:
                fillers.pop(0)()
        while pending:
            pending.pop(0)()
        run_all([emit_op3_final(m) for m in range(D // 128)])

        if debug:
            for p in range(NP):
                nc.sync.dma_start(d_qT[128 * p : 128 * (p + 1), :], qTt[p][:])
                nc.sync.dma_start(d_kT[128 * p : 128 * (p + 1), :], kTt[p][:])
                nc.sync.dma_start(d_ctxT[128 * p : 128 * (p + 1), :], ctxT[p][:])
            for st in range(ST):
                nc.sync.dma_start(d_v[128 * st : 128 * (st + 1), :], vt[st][:])

    nc.compile()
    return nc


_NC_CACHE = None


def kernel(x, Wq, Wk, Wv, Wo, bo):
    global _NC_CACHE
    if _NC_CACHE is None:
        _NC_CACHE = _build_nc()
    nc = _NC_CACHE

    bf = ml_dtypes.bfloat16
    tri = np.triu(np.ones((128, 128), dtype=np.float32)).astype(bf)
    in_maps = []
    for c in range(NC):
        b, hg = c // 2, c % 2
        cols = slice(DH * hg, DH * (hg + 1))
        in_maps.append(
            {
                "xT": np.ascontiguousarray(np.asarray(x)[b].T).astype(bf),
                "wq": np.asarray(Wq)[:, cols].astype(bf),
                "wk": np.asarray(Wk)[:, cols].astype(bf),
                "wv": np.asarray(Wv)[:, cols].astype(bf),
                "wo": np.asarray(Wo)[cols, :].astype(bf),
                "tri": tri,
            }
        )
    res = run_bass_kernel_spmd(nc, in_maps, core_ids=list(range(NC)))
    out = np.empty((B, S, D), dtype=np.float32)
    bo32 = np.asarray(bo, dtype=np.float32)
    for b in range(B):
        acc = res.results[2 * b]["outT"].astype(np.float32) + res.results[2 * b + 1][
            "outT"
        ].astype(np.float32)
        out[b] = acc.T + bo32
    return out


# revision 29
# speedup vs baseline: 1.0019x; 1.0019x over previous
"""Causal multi-head attention (B=4, S=2048, D=1024, H=16, HD=64) on 8 NeuronCores.

Sharding: core c handles batch b=c//2 and head-group hg=c%2 (8 heads each).
Each core computes out^T_partial = Wo_hg^T @ ctx_hg^T for its (b, hg); the host
sums the two head-group partials per batch, transposes, and adds the bias.

v2 structure (superblock-major, packed scores):
- Heads are processed in pairs (2p, 2p+1) living at SBUF partitions 0-63 /
  64-127 of qT/kT tile m=p.  The two heads' score matmuls (contraction K=64)
  are issued back-to-back at tile positions (0,0)/(64,0) so they run
  concurrently in disjoint row-groups of the PE array.
- Attention runs superblock-major (all pairs finish q-superblock I before
  I+1), with QKV-projection / out-projection matmuls statically interleaved
  between exp-dependent attention matmuls so the PE never waits on ScalarE.
- Softmax denominators come from a ones-column appended to V; normalization
  uses a reciprocal at partition 64 plus a K=1 broadcast matmul (no DMA).
- Output partials are written in bf16; the host sums in f32 and adds bias.
"""

import sys

for _p in ("/opt/trn_rl_repo",):
    if _p not in sys.path:
        sys.path.insert(0, _p)

import numpy as np
import ml_dtypes
from contextlib import ExitStack

import concourse.bacc as bacc
import concourse.tile as tile
from concourse import mybir
from concourse.bass_utils import run_bass_kernel_spmd

F32 = mybir.dt.float32
BF16 = mybir.dt.bfloat16
Exp = mybir.ActivationFunctionType.Exp
Mult = mybir.AluOpType.mult

B, S, D, H, HD = 4, 2048, 1024, 16, 64
NC = 8          # cores
HL = 8          # heads per core (head-group)
DH = HL * HD    # 512, per-core head dim
KT = D // 128   # 8 k-tiles over d_in
ST = S // 128   # 16 tiles over sequence
NB = S // 512   # 4 q-superblocks
NP = HL // 2    # 4 head pairs per core
SCALE = 1.0 / np.sqrt(HD)


def _build_nc(debug=False):
    nc = bacc.Bacc("TRN2", target_bir_lowering=False)

    xT = nc.declare_dram_parameter("xT", [D, S], BF16, isOutput=False)
    wq = nc.declare_dram_parameter("wq", [D, DH], BF16, isOutput=False)
    wk = nc.declare_dram_parameter("wk", [D, DH], BF16, isOutput=False)
    wv = nc.declare_dram_parameter("wv", [D, DH], BF16, isOutput=False)
    wo = nc.declare_dram_parameter("wo", [DH, D], BF16, isOutput=False)
    tri = nc.declare_dram_parameter("tri", [128, 128], BF16, isOutput=False)
    outT = nc.declare_dram_parameter("outT", [D, S], BF16, isOutput=True)
    if debug:
        d_qT = nc.declare_dram_parameter("d_qT", [DH, S], BF16, isOutput=True)
        d_kT = nc.declare_dram_parameter("d_kT", [DH, S], BF16, isOutput=True)
        d_v = nc.declare_dram_parameter("d_v", [S, HL * (HD + 1)], BF16, isOutput=True)
        d_ctxT = nc.declare_dram_parameter("d_ctxT", [DH, S], BF16, isOutput=True)

    with tile.TileContext(nc) as tc, ExitStack() as ctx:
        const_pool = ctx.enter_context(tc.tile_pool(name="const", bufs=1))
        x_pool = ctx.enter_context(tc.tile_pool(name="x", bufs=1))
        w_pool = ctx.enter_context(tc.tile_pool(name="w", bufs=1))
        qk_pool = ctx.enter_context(tc.tile_pool(name="qk", bufs=1))
        v_pool = ctx.enter_context(tc.tile_pool(name="v", bufs=1))
        ctxT_pool = ctx.enter_context(tc.tile_pool(name="ctxT", bufs=1))
        e_pool = ctx.enter_context(tc.tile_pool(name="e", bufs=4))
        r_pool = ctx.enter_context(tc.tile_pool(name="r", bufs=1))
        o_pool = ctx.enter_context(tc.tile_pool(name="o", bufs=2))
        o3_pool = ctx.enter_context(tc.tile_pool(name="o3", bufs=1))
        ps_sp = ctx.enter_context(tc.tile_pool(name="ps_sp", bufs=2, space="PSUM"))
        ps_c = ctx.enter_context(tc.tile_pool(name="ps_c", bufs=1, space="PSUM"))
        ps_gen = ctx.enter_context(tc.tile_pool(name="ps_gen", bufs=2, space="PSUM"))

        # ---- constants ----
        trit = const_pool.tile([128, 128], BF16)
        nc.sync.dma_start(trit[:], tri[:])
        onesb = const_pool.tile([1, 64], BF16)
        nc.vector.memset(onesb[:], 1.0)

        # ---- inputs ----
        # Per-k 2D slices (multi-dim dram APs under-synchronize on HW).
        # Triggers are spread across engine queues so the front-critical
        # loads (wv + x superblock 0) issue in parallel.
        wvt = w_pool.tile([128, KT, DH], BF16, name="wvt")
        wqt = w_pool.tile([128, KT, DH], BF16, name="wqt")
        wkt = w_pool.tile([128, KT, DH], BF16, name="wkt")
        wot = w_pool.tile([128, DH // 128, D], BF16, name="wot")
        xs = [x_pool.tile([128, KT, 512], BF16, name=f"xs{_}") for _ in range(NB)]

        # Priority order on one queue: a DMA's descriptors go out before the
        # next trigger's, so the front-critical tensors finish first instead
        # of fair-sharing HBM bandwidth with the whole load set.
        def xsrc(s):
            return xT[:, 512 * s : 512 * (s + 1)].rearrange("(k p) c -> p k c", p=128)

        for k in range(KT):
            nc.sync.dma_start(wvt[:, k, :], wv[128 * k : 128 * (k + 1), :])
            nc.sync.dma_start(xs[0][:, k, :], xT[128 * k : 128 * (k + 1), 0:512])
        nc.sync.dma_start(wqt[:], wq.rearrange("(k p) c -> p k c", p=128)[:])
        nc.sync.dma_start(wkt[:], wk.rearrange("(k p) c -> p k c", p=128)[:])
        nc.sync.dma_start(xs[1][:], xsrc(1)[:])
        nc.sync.dma_start(xs[2][:], xsrc(2)[:])
        nc.sync.dma_start(xs[3][:], xsrc(3)[:])
        nc.sync.dma_start(wot[:], wo.rearrange("(k p) c -> p k c", p=128)[:])

        # ---- persistent activations ----
        qTt = [qk_pool.tile([128, S], BF16, name=f"qT{_}") for _ in range(NP)]
        kTt = [qk_pool.tile([128, S], BF16, name=f"kT{_}") for _ in range(NP)]
        vt = [v_pool.tile([128, HL * (HD + 1)], BF16, name=f"v{_}") for _ in range(ST)]
        ctxT = [ctxT_pool.tile([128, S], BF16, name=f"ctxT{_}") for _ in range(NP)]

        # ---- generation units ----
        # Each unit is one PSUM accumulation chain, split into two half-unit
        # closures (~4 matmuls each) for fine-grained interleaving with the
        # attention rounds.  Interleaving foreign matmuls inside an open
        # accumulation group is legal (per-bank has_written state).
        def emit_v(st):
            """V for k-tile st: [128, 8*(HD+1)] with a ones column per head."""
            state = {}
            xcol = xs[st // 4]

            def half_a():
                nc.vector.memset(
                    vt[st].rearrange("p (h c) -> p h c", c=HD + 1)[:, :, HD], 1.0
                )
                state["pv"] = ps_gen.tile([128, 512], F32, tag="pg", name=f"pv{st}")
                for k in range(KT // 2):
                    nc.tensor.matmul(
                        state["pv"][:],
                        xcol[:, k, 128 * (st % 4) : 128 * (st % 4 + 1)],
                        wvt[:, k, :],
                        start=(k == 0),
                        stop=False,
                    )

            def half_b():
                pv = state["pv"]
                for k in range(KT // 2, KT):
                    nc.tensor.matmul(
                        pv[:],
                        xcol[:, k, 128 * (st % 4) : 128 * (st % 4 + 1)],
                        wvt[:, k, :],
                        start=False,
                        stop=(k == KT - 1),
                    )
                nc.vector.tensor_copy(
                    vt[st].rearrange("p (h c) -> p h c", c=HD + 1)[:, :, 0:HD],
                    pv.rearrange("p (h c) -> p h c", c=HD)[:],
                )

            return [half_a, half_b]

        def emit_qk(wt, dst, m, n):
            """q^T or k^T for head-pair m, sequence superblock n."""
            state = {}

            def half_a():
                state["ps"] = ps_gen.tile([128, 512], F32, tag="pg", name=f"pqk{m}_{n}")
                for k in range(KT // 2):
                    nc.tensor.matmul(
                        state["ps"][:],
                        wt[:, k, 128 * m : 128 * (m + 1)],
                        xs[n][:, k, :],
                        start=(k == 0),
                        stop=False,
                    )

            def half_b():
                ps = state["ps"]
                for k in range(KT // 2, KT):
                    nc.tensor.matmul(
                        ps[:],
                        wt[:, k, 128 * m : 128 * (m + 1)],
                        xs[n][:, k, :],
                        start=False,
                        stop=(k == KT - 1),
                    )
                nc.vector.tensor_copy(dst[m][:, 512 * n : 512 * (n + 1)], ps[:])

            return [half_a, half_b]

        def emit_op(m, n):
            """out^T rows [128m:128(m+1)], columns superblock n."""

            def unit():
                ps = ps_gen.tile([128, 512], F32, tag="pg", name=f"pop{m}_{n}")
                for k in range(DH // 128):
                    nc.tensor.matmul(
                        ps[:],
                        wot[:, k, 128 * m : 128 * (m + 1)],
                        ctxT[k][:, 512 * n : 512 * (n + 1)],
                        start=(k == 0),
                        stop=(k == DH // 128 - 1),
                    )
                ot = o_pool.tile([128, 512], BF16, tag="ot", name=f"ot{m}_{n}")
                nc.vector.tensor_copy(ot[:], ps[:])
                nc.sync.dma_start(
                    outT[128 * m : 128 * (m + 1), 512 * n : 512 * (n + 1)], ot[:]
                )

            return [unit]

        # Final column-superblock out-proj, split so only a rank-128 update
        # plus an add remains after the last head finishes.
        o3_tiles = {}

        def emit_op3_partial(m):
            def unit():
                ps = ps_gen.tile([128, 512], F32, tag="pg", name=f"pop3a{m}")
                for k in range(3):
                    nc.tensor.matmul(
                        ps[:],
                        wot[:, k, 128 * m : 128 * (m + 1)],
                        ctxT[k][:, 1536:2048],
                        start=(k == 0),
                        stop=(k == 2),
                    )
                t = o3_pool.tile([128, 512], F32, tag=f"o3_{m}", name=f"o3_{m}")
                nc.vector.tensor_copy(t[:], ps[:])
                o3_tiles[m] = t

            return [unit]

        def emit_op3_final(m):
            def unit():
                ps = ps_gen.tile([128, 512], F32, tag="pg", name=f"pop3b{m}")
                nc.tensor.matmul(
                    ps[:],
                    wot[:, 3, 128 * m : 128 * (m + 1)],
                    ctxT[3][:, 1536:2048],
                    start=True,
                    stop=True,
                )
                ot = o_pool.tile([128, 512], BF16, tag="ot", name=f"ot3_{m}")
                nc.vector.tensor_tensor(
                    ot[:], o3_tiles[m][:], ps[:], mybir.AluOpType.add
                )
                nc.sync.dma_start(outT[128 * m : 128 * (m + 1), 1536:2048], ot[:])

            return [unit]

        # ---- attention ----
        pending = []  # deferred normalization closures

        def make_norm(p, I, X, cps):
            def _norm():
                cun = r_pool.tile([65, 512], F32, tag=f"cun{X}", name="cun")
                nc.vector.tensor_copy(cun[:], cps[:])
                # den row must move to partition 0: reciprocal_approx_fast is
                # custom DVE ucode and corrupts SBUF at a nonzero base
                # partition (HW-only; CoreSim doesn't model it).
                den0 = r_pool.tile([1, 512], F32, tag="den0", name="den0")
                nc.sync.dma_start(den0[0:1, :], cun[64:65, :])
                rec = r_pool.tile([1, 512], F32, tag="rec", name="rec")
                nc.vector.reciprocal_approx_fast(rec[0:1, :], den0[0:1, :])
                recb = r_pool.tile([1, 512], BF16, tag="recb", name="recb")
                nc.vector.tensor_copy(recb[0:1, :], rec[0:1, :])
                bc = ps_c.tile([65, 512], F32, tag=f"c{X}", name="bc")
                nc.tensor.matmul(
                    bc[0:64, :], onesb[0:1, 0:64], recb[0:1, :],
                    start=True, stop=True,
                )
                dst = ctxT[p][64 * X : 64 * X + 64, 512 * I : 512 * (I + 1)]
                if X == 0:
                    nc.vector.tensor_tensor(dst, cun[0:64, :], bc[0:64, :], Mult)
                else:
                    nrm = r_pool.tile([64, 512], BF16, tag="nrm", name="nrm")
                    nc.vector.tensor_tensor(nrm[:], cun[0:64, :], bc[0:64, :], Mult)
                    nc.sync.dma_start(dst, nrm[:])

            return _norm

        def attn_pair(p, I, fillers, budget, urgent=None):
            """All k-rounds for head pair p, q-superblock I.

            fillers: deque of half-unit closures; budget[0] accumulates the
            fill pacing fraction per round.  urgent: closures popped one per
            round ahead of the budgeted fillers (deadline-critical units)."""
            nj = 4 * I + 4
            cps = [
                ps_c.tile([65, 512], F32, tag=f"c{X}", name=f"cps{X}")
                for X in range(2)
            ]
            for j in range(nj):
                diag = (j // 4 == I)
                lo = 128 * (j - 4 * I) if diag else 0
                sp = ps_sp.tile([128, 1024], F32, tag="sp", name="sp")
                if lo > 0:
                    # B-head's masked hole would be uninitialized PSUM under
                    # the single merged exp below.
                    nc.vector.memset(sp[:, 512 : 512 + lo], 0.0)
                for X in range(2):
                    nc.tensor.matmul(
                        sp[:, 512 * X + lo : 512 * (X + 1)],
                        kTt[p][64 * X : 64 * X + 64, 128 * j : 128 * (j + 1)],
                        qTt[p][64 * X : 64 * X + 64, 512 * I + lo : 512 * (I + 1)],
                        start=True,
                        stop=True,
                    )
                e = e_pool.tile([128, 1024], BF16, tag="e", name="e")
                nc.scalar.activation(
                    e[:, lo:1024], sp[:, lo:1024], Exp, scale=float(SCALE)
                )
                if diag:
                    for X in range(2):
                        nc.vector.tensor_tensor(
                            e[:, 512 * X + lo : 512 * X + lo + 128],
                            e[:, 512 * X + lo : 512 * X + lo + 128],
                            trit[:],
                            Mult,
                        )
                while pending:
                    pending.pop(0)()
                if urgent:
                    urgent.pop(0)()
                budget[0] += budget[1]
                while budget[0] >= 1.0 and fillers:
                    fillers.pop(0)()
                    budget[0] -= 1.0
                for X in range(2):
                    nc.tensor.matmul(
                        cps[X][:, lo:512],
                        vt[j][:, (HD + 1) * (2 * p + X) : (HD + 1) * (2 * p + X + 1)],
                        e[:, 512 * X + lo : 512 * (X + 1)],
                        start=(j == 0),
                        stop=(j == nj - 1),
                        skip_group_check=True,
                    )
            for X in range(2):
                pending.append(make_norm(p, I, X, cps[X]))

        # ---- emission schedule ----
        def run_all(units):
            for u in units:
                for half in u:
                    half()

        def flat(units):
            return [half for u in units for half in u]

        # upfront: V k-tiles 0-3 and q/k for pair 0, superblock 0
        run_all([emit_v(st) for st in range(4)])
        run_all([emit_qk(wqt, qTt, 0, 0), emit_qk(wkt, kTt, 0, 0)])

        phase_fillers = [
            # during sb0: remaining sb0 q/k, V 4-7, all of sb1 q/k
            flat(
                [emit_qk(wqt, qTt, m, 0) for m in range(1, NP)]
                + [emit_qk(wkt, kTt, m, 0) for m in range(1, NP)]
                + [emit_v(st) for st in range(4, 8)]
                + [emit_qk(wqt, qTt, m, 1) for m in range(NP)]
                + [emit_qk(wkt, kTt, m, 1) for m in range(NP)]
            ),
            # during sb1: sb2 q/k
            flat(
                [emit_qk(wqt, qTt, m, 2) for m in range(NP)]
                + [emit_qk(wkt, kTt, m, 2) for m in range(NP)]
            ),
            # during sb2: sb3 q/k (V 8-11 in the urgent lane)
            flat(
                [emit_qk(wqt, qTt, m, 3) for m in range(NP)]
                + [emit_qk(wkt, kTt, m, 3) for m in range(NP)]
            ),
            # during sb3: all deferrable out-proj columns (V 12-15 in the
            # urgent lane: needed by round 12 = pair 0's j=12)
            flat(
                [emit_op(m, 0) for m in range(D // 128)]
                + [emit_op(m, 1) for m in range(D // 128)]
                + [emit_op(m, 2) for m in range(D // 128)]
            ),
        ]
        # urgent lanes, popped one per round ahead of budgeted fillers
        # (deadline-critical V generation for the next superblock's k-tiles)
        urgent_lanes = {
            (2, 0): flat([emit_v(st) for st in range(8, 12)]),
            (3, 0): flat([emit_v(st) for st in range(12, 16)]),
        }

        for I in range(NB):
            fillers = phase_fillers[I]
            rounds = NP * (4 * I + 4)
            budget = [0.999, len(fillers) / rounds]
            for p in range(NP):
                urgent = urgent_lanes.get((I, p), [])
                attn_pair(p, I, fillers, budget, urgent)
                while urgent:
                    urgent.pop(0)()
            while fillers:
                fillers.pop(0)()
        while pending:
            pending.pop(0)()
        run_all([emit_op(m, 3) for m in range(D // 128)])

        if debug:
            for p in range(NP):
                nc.sync.dma_start(d_qT[128 * p : 128 * (p + 1), :], qTt[p][:])
                nc.sync.dma_start(d_kT[128 * p : 128 * (p + 1), :], kTt[p][:])
                nc.sync.dma_start(d_ctxT[128 * p : 128 * (p + 1), :], ctxT[p][:])
            for st in range(ST):
                nc.sync.dma_start(d_v[128 * st : 128 * (st + 1), :], vt[st][:])

    nc.compile()
    return nc


_NC_CACHE = None


def kernel(x, Wq, Wk, Wv, Wo, bo):
    global _NC_CACHE
    if _NC_CACHE is None:
        _NC_CACHE = _build_nc()
    nc = _NC_CACHE

    bf = ml_dtypes.bfloat16
    tri = np.triu(np.ones((128, 128), dtype=np.float32)).astype(bf)
    in_maps = []
    for c in range(NC):
        b, hg = c // 2, c % 2
        cols = slice(DH * hg, DH * (hg + 1))
        in_maps.append(
            {
                "xT": np.ascontiguousarray(np.asarray(x)[b].T).astype(bf),
                "wq": np.asarray(Wq)[:, cols].astype(bf),
                "wk": np.asarray(Wk)[:, cols].astype(bf),
                "wv": np.asarray(Wv)[:, cols].astype(bf),
                "wo": np.asarray(Wo)[cols, :].astype(bf),
                "tri": tri,
            }
        )
    res = run_bass_kernel_spmd(nc, in_maps, core_ids=list(range(NC)))
    out = np.empty((B, S, D), dtype=np.float32)
    bo32 = np.asarray(bo, dtype=np.float32)
    for b in range(B):
        acc = res.results[2 * b]["outT"].astype(np.float32) + res.results[2 * b + 1][
            "outT"
        ].astype(np.float32)
        out[b] = acc.T + bo32
    return out


# revision 31
# speedup vs baseline: 1.0852x; 1.0831x over previous
"""Causal multi-head attention (B=4, S=2048, D=1024, H=16, HD=64) on 8 NeuronCores.

Sharding: core c handles batch b=c//2 and head-group hg=c%2 (8 heads each).
Each core computes out^T_partial = Wo_hg^T @ ctx_hg^T for its (b, hg); the host
sums the two head-group partials per batch, transposes, and adds the bias.

v2 structure (superblock-major, packed scores):
- Heads are processed in pairs (2p, 2p+1) living at SBUF partitions 0-63 /
  64-127 of qT/kT tile m=p.  The two heads' score matmuls (contraction K=64)
  are issued back-to-back at tile positions (0,0)/(64,0) so they run
  concurrently in disjoint row-groups of the PE array.
- Attention runs superblock-major (all pairs finish q-superblock I before
  I+1), with QKV-projection / out-projection matmuls statically interleaved
  between exp-dependent attention matmuls so the PE never waits on ScalarE.
- Softmax denominators come from a ones-column appended to V; normalization
  uses a reciprocal at partition 64 plus a K=1 broadcast matmul (no DMA).
- Output partials are written in bf16; the host sums in f32 and adds bias.
"""

import sys

for _p in ("/opt/trn_rl_repo",):
    if _p not in sys.path:
        sys.path.insert(0, _p)

import numpy as np
import ml_dtypes
from contextlib import ExitStack

import concourse.bacc as bacc
import concourse.tile as tile
from concourse import mybir
from concourse.bass_utils import run_bass_kernel_spmd

F32 = mybir.dt.float32
BF16 = mybir.dt.bfloat16
Exp = mybir.ActivationFunctionType.Exp
Mult = mybir.AluOpType.mult

B, S, D, H, HD = 4, 2048, 1024, 16, 64
NC = 8          # cores
HL = 8          # heads per core (head-group)
DH = HL * HD    # 512, per-core head dim
KT = D // 128   # 8 k-tiles over d_in
ST = S // 128   # 16 tiles over sequence
NB = S // 512   # 4 q-superblocks
NP = HL // 2    # 4 head pairs per core
SCALE = 1.0 / np.sqrt(HD)


def _build_nc(debug=False):
    nc = bacc.Bacc("TRN2", target_bir_lowering=False)

    xT = nc.declare_dram_parameter("xT", [D, S], BF16, isOutput=False)
    wq = nc.declare_dram_parameter("wq", [D, DH], BF16, isOutput=False)
    wk = nc.declare_dram_parameter("wk", [D, DH], BF16, isOutput=False)
    wv = nc.declare_dram_parameter("wv", [D, DH], BF16, isOutput=False)
    wo = nc.declare_dram_parameter("wo", [DH, D], BF16, isOutput=False)
    tri = nc.declare_dram_parameter("tri", [128, 128], BF16, isOutput=False)
    outT = nc.declare_dram_parameter("outT", [D, S], BF16, isOutput=True)
    if debug:
        d_qT = nc.declare_dram_parameter("d_qT", [DH, S], BF16, isOutput=True)
        d_kT = nc.declare_dram_parameter("d_kT", [DH, S], BF16, isOutput=True)
        d_v = nc.declare_dram_parameter("d_v", [S, HL * (HD + 1)], BF16, isOutput=True)
        d_ctxT = nc.declare_dram_parameter("d_ctxT", [DH, S], BF16, isOutput=True)

    with tile.TileContext(nc) as tc, ExitStack() as ctx:
        const_pool = ctx.enter_context(tc.tile_pool(name="const", bufs=1))
        x_pool = ctx.enter_context(tc.tile_pool(name="x", bufs=1))
        w_pool = ctx.enter_context(tc.tile_pool(name="w", bufs=1))
        qk_pool = ctx.enter_context(tc.tile_pool(name="qk", bufs=1))
        v_pool = ctx.enter_context(tc.tile_pool(name="v", bufs=1))
        ctxT_pool = ctx.enter_context(tc.tile_pool(name="ctxT", bufs=1))
        e_pool = ctx.enter_context(tc.tile_pool(name="e", bufs=6))
        r_pool = ctx.enter_context(tc.tile_pool(name="r", bufs=1))
        o_pool = ctx.enter_context(tc.tile_pool(name="o", bufs=2))
        o3_pool = ctx.enter_context(tc.tile_pool(name="o3", bufs=1))
        ps_sp = ctx.enter_context(tc.tile_pool(name="ps_sp", bufs=2, space="PSUM"))
        ps_c = ctx.enter_context(tc.tile_pool(name="ps_c", bufs=1, space="PSUM"))
        ps_gen = ctx.enter_context(tc.tile_pool(name="ps_gen", bufs=2, space="PSUM"))

        # ---- constants ----
        trit = const_pool.tile([128, 128], BF16)
        nc.sync.dma_start(trit[:], tri[:])
        onesb = const_pool.tile([1, 64], BF16)
        nc.vector.memset(onesb[:], 1.0)

        # ---- inputs ----
        # Per-k 2D slices (multi-dim dram APs under-synchronize on HW).
        # Triggers are spread across engine queues so the front-critical
        # loads (wv + x superblock 0) issue in parallel.
        wvt = w_pool.tile([128, KT, DH], BF16, name="wvt")
        wqt = w_pool.tile([128, KT, DH], BF16, name="wqt")
        wkt = w_pool.tile([128, KT, DH], BF16, name="wkt")
        wot = w_pool.tile([128, DH // 128, D], BF16, name="wot")
        xs = [x_pool.tile([128, KT, 512], BF16, name=f"xs{_}") for _ in range(NB)]

        # Priority order on one queue: a DMA's descriptors go out before the
        # next trigger's, so the front-critical tensors finish first instead
        # of fair-sharing HBM bandwidth with the whole load set.
        def xsrc(s):
            return xT[:, 512 * s : 512 * (s + 1)].rearrange("(k p) c -> p k c", p=128)

        for k in range(KT):
            nc.sync.dma_start(wvt[:, k, :], wv[128 * k : 128 * (k + 1), :])
            nc.sync.dma_start(xs[0][:, k, :], xT[128 * k : 128 * (k + 1), 0:512])
        nc.sync.dma_start(wqt[:], wq.rearrange("(k p) c -> p k c", p=128)[:])
        nc.sync.dma_start(wkt[:], wk.rearrange("(k p) c -> p k c", p=128)[:])
        nc.sync.dma_start(xs[1][:], xsrc(1)[:])
        nc.sync.dma_start(xs[2][:], xsrc(2)[:])
        nc.sync.dma_start(xs[3][:], xsrc(3)[:])
        nc.sync.dma_start(wot[:], wo.rearrange("(k p) c -> p k c", p=128)[:])

        # ---- persistent activations ----
        qTt = [qk_pool.tile([128, S], BF16, name=f"qT{_}") for _ in range(NP)]
        kTt = [qk_pool.tile([128, S], BF16, name=f"kT{_}") for _ in range(NP)]
        vt = [v_pool.tile([128, HL * (HD + 1)], BF16, name=f"v{_}") for _ in range(ST)]
        ctxT = [ctxT_pool.tile([128, S], BF16, name=f"ctxT{_}") for _ in range(NP)]

        # ---- generation units ----
        # Each unit is one PSUM accumulation chain, split into two half-unit
        # closures (~4 matmuls each) for fine-grained interleaving with the
        # attention rounds.  Interleaving foreign matmuls inside an open
        # accumulation group is legal (per-bank has_written state).
        def emit_v(st):
            """V for k-tile st: [128, 8*(HD+1)] with a ones column per head."""
            state = {}
            xcol = xs[st // 4]

            def half_a():
                nc.vector.memset(
                    vt[st].rearrange("p (h c) -> p h c", c=HD + 1)[:, :, HD], 1.0
                )
                state["pv"] = ps_gen.tile([128, 512], F32, tag="pg", name=f"pv{st}")
                for k in range(KT // 2):
                    nc.tensor.matmul(
                        state["pv"][:],
                        xcol[:, k, 128 * (st % 4) : 128 * (st % 4 + 1)],
                        wvt[:, k, :],
                        start=(k == 0),
                        stop=False,
                    )

            def half_b():
                pv = state["pv"]
                for k in range(KT // 2, KT):
                    nc.tensor.matmul(
                        pv[:],
                        xcol[:, k, 128 * (st % 4) : 128 * (st % 4 + 1)],
                        wvt[:, k, :],
                        start=False,
                        stop=(k == KT - 1),
                    )
                nc.vector.tensor_copy(
                    vt[st].rearrange("p (h c) -> p h c", c=HD + 1)[:, :, 0:HD],
                    pv.rearrange("p (h c) -> p h c", c=HD)[:],
                )

            return [half_a, half_b]

        def emit_qk(wt, dst, m, n):
            """q^T or k^T for head-pair m, sequence superblock n."""
            state = {}

            def half_a():
                state["ps"] = ps_gen.tile([128, 512], F32, tag="pg", name=f"pqk{m}_{n}")
                for k in range(KT // 2):
                    nc.tensor.matmul(
                        state["ps"][:],
                        wt[:, k, 128 * m : 128 * (m + 1)],
                        xs[n][:, k, :],
                        start=(k == 0),
                        stop=False,
                    )

            def half_b():
                ps = state["ps"]
                for k in range(KT // 2, KT):
                    nc.tensor.matmul(
                        ps[:],
                        wt[:, k, 128 * m : 128 * (m + 1)],
                        xs[n][:, k, :],
                        start=False,
                        stop=(k == KT - 1),
                    )
                nc.vector.tensor_copy(dst[m][:, 512 * n : 512 * (n + 1)], ps[:])

            return [half_a, half_b]

        def emit_op(m, n):
            """out^T rows [128m:128(m+1)], columns superblock n."""
            state = {}

            def half_a():
                state["ps"] = ps_gen.tile([128, 512], F32, tag="pg", name=f"pop{m}_{n}")
                for k in range(2):
                    nc.tensor.matmul(
                        state["ps"][:],
                        wot[:, k, 128 * m : 128 * (m + 1)],
                        ctxT[k][:, 512 * n : 512 * (n + 1)],
                        start=(k == 0),
                        stop=False,
                    )

            def half_b():
                ps = state["ps"]
                for k in range(2, DH // 128):
                    nc.tensor.matmul(
                        ps[:],
                        wot[:, k, 128 * m : 128 * (m + 1)],
                        ctxT[k][:, 512 * n : 512 * (n + 1)],
                        start=False,
                        stop=(k == DH // 128 - 1),
                    )
                ot = o_pool.tile([128, 512], BF16, tag="ot", name=f"ot{m}_{n}")
                nc.vector.tensor_copy(ot[:], ps[:])
                nc.sync.dma_start(
                    outT[128 * m : 128 * (m + 1), 512 * n : 512 * (n + 1)], ot[:]
                )

            return [half_a, half_b]

        # Final column-superblock out-proj, split so only a rank-128 update
        # plus an add remains after the last head finishes.
        o3_tiles = {}

        def emit_op3_partial(m):
            def unit():
                ps = ps_gen.tile([128, 512], F32, tag="pg", name=f"pop3a{m}")
                for k in range(3):
                    nc.tensor.matmul(
                        ps[:],
                        wot[:, k, 128 * m : 128 * (m + 1)],
                        ctxT[k][:, 1536:2048],
                        start=(k == 0),
                        stop=(k == 2),
                    )
                t = o3_pool.tile([128, 512], F32, tag=f"o3_{m}", name=f"o3_{m}")
                nc.vector.tensor_copy(t[:], ps[:])
                o3_tiles[m] = t

            return [unit]

        def emit_op3_final(m):
            def unit():
                ps = ps_gen.tile([128, 512], F32, tag="pg", name=f"pop3b{m}")
                nc.tensor.matmul(
                    ps[:],
                    wot[:, 3, 128 * m : 128 * (m + 1)],
                    ctxT[3][:, 1536:2048],
                    start=True,
                    stop=True,
                )
                ot = o_pool.tile([128, 512], BF16, tag="ot", name=f"ot3_{m}")
                nc.vector.tensor_tensor(
                    ot[:], o3_tiles[m][:], ps[:], mybir.AluOpType.add
                )
                nc.sync.dma_start(outT[128 * m : 128 * (m + 1), 1536:2048], ot[:])

            return [unit]

        # ---- attention ----
        pending = []  # deferred normalization closures

        def make_norm(p, I, X, cps):
            def _norm():
                cun = r_pool.tile([65, 512], F32, tag=f"cun{X}", name="cun")
                nc.vector.tensor_copy(cun[:], cps[:])
                # den row must move to partition 0: reciprocal_approx_fast is
                # custom DVE ucode and corrupts SBUF at a nonzero base
                # partition (HW-only; CoreSim doesn't model it).
                den0 = r_pool.tile([1, 512], F32, tag="den0", name="den0")
                nc.sync.dma_start(den0[0:1, :], cun[64:65, :])
                rec = r_pool.tile([1, 512], F32, tag="rec", name="rec")
                nc.vector.reciprocal_approx_fast(rec[0:1, :], den0[0:1, :])
                recb = r_pool.tile([1, 512], BF16, tag="recb", name="recb")
                nc.vector.tensor_copy(recb[0:1, :], rec[0:1, :])
                bc = ps_c.tile([65, 512], F32, tag=f"c{X}", name="bc")
                nc.tensor.matmul(
                    bc[0:64, :], onesb[0:1, 0:64], recb[0:1, :],
                    start=True, stop=True,
                )
                dst = ctxT[p][64 * X : 64 * X + 64, 512 * I : 512 * (I + 1)]
                if X == 0:
                    nc.vector.tensor_tensor(dst, cun[0:64, :], bc[0:64, :], Mult)
                else:
                    nrm = r_pool.tile([64, 512], BF16, tag="nrm", name="nrm")
                    nc.vector.tensor_tensor(nrm[:], cun[0:64, :], bc[0:64, :], Mult)
                    nc.sync.dma_start(dst, nrm[:])

            return _norm

        def attn_pair(p, I, fillers, budget, urgent=None):
            """All k-rounds for head pair p, q-superblock I.

            fillers: deque of half-unit closures; budget[0] accumulates the
            fill pacing fraction per round.  urgent: closures popped one per
            round ahead of the budgeted fillers (deadline-critical units)."""
            nj = 4 * I + 4
            cps = [
                ps_c.tile([65, 512], F32, tag=f"c{X}", name=f"cps{X}")
                for X in range(2)
            ]
            for j in range(nj):
                diag = (j // 4 == I)
                lo = 128 * (j - 4 * I) if diag else 0
                sp = ps_sp.tile([128, 1024], F32, tag="sp", name="sp")
                if lo > 0:
                    # B-head's masked hole would be uninitialized PSUM under
                    # the single merged exp below.
                    nc.vector.memset(sp[:, 512 : 512 + lo], 0.0)
                for X in range(2):
                    nc.tensor.matmul(
                        sp[:, 512 * X + lo : 512 * (X + 1)],
                        kTt[p][64 * X : 64 * X + 64, 128 * j : 128 * (j + 1)],
                        qTt[p][64 * X : 64 * X + 64, 512 * I + lo : 512 * (I + 1)],
                        start=True,
                        stop=True,
                    )
                e = e_pool.tile([128, 1024], BF16, tag="e", name="e")
                nc.scalar.activation(
                    e[:, lo:1024], sp[:, lo:1024], Exp, scale=float(SCALE)
                )
                if diag:
                    for X in range(2):
                        nc.vector.tensor_tensor(
                            e[:, 512 * X + lo : 512 * X + lo + 128],
                            e[:, 512 * X + lo : 512 * X + lo + 128],
                            trit[:],
                            Mult,
                        )
                while pending:
                    pending.pop(0)()
                if urgent:
                    urgent.pop(0)()
                budget[0] += budget[1]
                while budget[0] >= 1.0 and fillers:
                    fillers.pop(0)()
                    budget[0] -= 1.0
                for X in range(2):
                    nc.tensor.matmul(
                        cps[X][:, lo:512],
                        vt[j][:, (HD + 1) * (2 * p + X) : (HD + 1) * (2 * p + X + 1)],
                        e[:, 512 * X + lo : 512 * (X + 1)],
                        start=(j == 0),
                        stop=(j == nj - 1),
                        skip_group_check=True,
                    )
            for X in range(2):
                pending.append(make_norm(p, I, X, cps[X]))

        # ---- emission schedule ----
        def run_all(units):
            for u in units:
                for half in u:
                    half()

        def flat(units):
            return [half for u in units for half in u]

        # upfront: V k-tiles 0-3 and q/k for pair 0, superblock 0
        run_all([emit_v(st) for st in range(4)])
        run_all([emit_qk(wqt, qTt, 0, 0), emit_qk(wkt, kTt, 0, 0)])

        phase_fillers = [
            # during sb0: remaining sb0 q/k, V 4-7, all of sb1 q/k
            flat(
                [emit_qk(wqt, qTt, m, 0) for m in range(1, NP)]
                + [emit_qk(wkt, kTt, m, 0) for m in range(1, NP)]
                + [emit_v(st) for st in range(4, 8)]
                + [emit_qk(wqt, qTt, m, 1) for m in range(NP)]
                + [emit_qk(wkt, kTt, m, 1) for m in range(NP)]
            ),
            # during sb1: sb2 q/k
            flat(
                [emit_qk(wqt, qTt, m, 2) for m in range(NP)]
                + [emit_qk(wkt, kTt, m, 2) for m in range(NP)]
            ),
            # during sb2: sb3 q/k (V 8-11 in the urgent lane)
            flat(
                [emit_qk(wqt, qTt, m, 3) for m in range(NP)]
                + [emit_qk(wkt, kTt, m, 3) for m in range(NP)]
            ),
            # during sb3: all deferrable out-proj columns (V 12-15 in the
            # urgent lane: needed by round 12 = pair 0's j=12)
            flat(
                [emit_op(m, 0) for m in range(D // 128)]
                + [emit_op(m, 1) for m in range(D // 128)]
                + [emit_op(m, 2) for m in range(D // 128)]
            ),
        ]
        # urgent lanes, popped one per round ahead of budgeted fillers
        # (deadline-critical V generation for the next superblock's k-tiles)
        urgent_lanes = {
            (2, 0): flat([emit_v(st) for st in range(8, 12)]),
            (3, 0): flat([emit_v(st) for st in range(12, 16)]),
        }

        for I in range(NB):
            fillers = phase_fillers[I]
            rounds = NP * (4 * I + 4)
            budget = [0.999, len(fillers) / rounds]
            for p in range(NP):
                urgent = urgent_lanes.get((I, p), [])
                attn_pair(p, I, fillers, budget, urgent)
                while urgent:
                    urgent.pop(0)()
            while fillers:
                fillers.pop(0)()
        while pending:
            pending.pop(0)()
        run_all([emit_op(m, 3) for m in range(D // 128)])

        if debug:
            for p in range(NP):
                nc.sync.dma_start(d_qT[128 * p : 128 * (p + 1), :], qTt[p][:])
                nc.sync.dma_start(d_kT[128 * p : 128 * (p + 1), :], kTt[p][:])
                nc.sync.dma_start(d_ctxT[128 * p : 128 * (p + 1), :], ctxT[p][:])
            for st in range(ST):
                nc.sync.dma_start(d_v[128 * st : 128 * (st + 1), :], vt[st][:])

    nc.compile()
    return nc


_NC_CACHE = None


def kernel(x, Wq, Wk, Wv, Wo, bo):
    global _NC_CACHE
    if _NC_CACHE is None:
        _NC_CACHE = _build_nc()
    nc = _NC_CACHE

    bf = ml_dtypes.bfloat16
    tri = np.triu(np.ones((128, 128), dtype=np.float32)).astype(bf)
    in_maps = []
    for c in range(NC):
        b, hg = c // 2, c % 2
        cols = slice(DH * hg, DH * (hg + 1))
        in_maps.append(
            {
                "xT": np.ascontiguousarray(np.asarray(x)[b].T).astype(bf),
                "wq": np.asarray(Wq)[:, cols].astype(bf),
                "wk": np.asarray(Wk)[:, cols].astype(bf),
                "wv": np.asarray(Wv)[:, cols].astype(bf),
                "wo": np.asarray(Wo)[cols, :].astype(bf),
                "tri": tri,
            }
        )
    res = run_bass_kernel_spmd(nc, in_maps, core_ids=list(range(NC)))
    out = np.empty((B, S, D), dtype=np.float32)
    bo32 = np.asarray(bo, dtype=np.float32)
    for b in range(B):
        acc = res.results[2 * b]["outT"].astype(np.float32) + res.results[2 * b + 1][
            "outT"
        ].astype(np.float32)
        out[b] = acc.T + bo32
    return out


# revision 33
# speedup vs baseline: 1.1410x; 1.0515x over previous
"""Causal multi-head attention (B=4, S=2048, D=1024, H=16, HD=64) on 8 NeuronCores.

Sharding: core c handles batch b=c//2 and head-group hg=c%2 (8 heads each).
Each core computes out^T_partial = Wo_hg^T @ ctx_hg^T for its (b, hg); the host
sums the two head-group partials per batch, transposes, and adds the bias.

v2 structure (superblock-major, packed scores):
- Heads are processed in pairs (2p, 2p+1) living at SBUF partitions 0-63 /
  64-127 of qT/kT tile m=p.  The two heads' score matmuls (contraction K=64)
  are issued back-to-back at tile positions (0,0)/(64,0) so they run
  concurrently in disjoint row-groups of the PE array.
- Attention runs superblock-major (all pairs finish q-superblock I before
  I+1), with QKV-projection / out-projection matmuls statically interleaved
  between exp-dependent attention matmuls so the PE never waits on ScalarE.
- Softmax denominators come from a ones-column appended to V; normalization
  uses a reciprocal at partition 64 plus a K=1 broadcast matmul (no DMA).
- Output partials are written in bf16; the host sums in f32 and adds bias.
"""

import sys

for _p in ("/opt/trn_rl_repo",):
    if _p not in sys.path:
        sys.path.insert(0, _p)

import numpy as np
import ml_dtypes
from contextlib import ExitStack

import concourse.bacc as bacc
import concourse.tile as tile
from concourse import mybir
from concourse.bass_utils import run_bass_kernel_spmd

F32 = mybir.dt.float32
BF16 = mybir.dt.bfloat16
Exp = mybir.ActivationFunctionType.Exp
Mult = mybir.AluOpType.mult

B, S, D, H, HD = 4, 2048, 1024, 16, 64
NC = 8          # cores
HL = 8          # heads per core (head-group)
DH = HL * HD    # 512, per-core head dim
KT = D // 128   # 8 k-tiles over d_in
ST = S // 128   # 16 tiles over sequence
NB = S // 512   # 4 q-superblocks
NP = HL // 2    # 4 head pairs per core
SCALE = 1.0 / np.sqrt(HD)


def _build_nc(debug=False):
    nc = bacc.Bacc("TRN2", target_bir_lowering=False)

    xT = nc.declare_dram_parameter("xT", [D, S], BF16, isOutput=False)
    wq = nc.declare_dram_parameter("wq", [D, DH], BF16, isOutput=False)
    wk = nc.declare_dram_parameter("wk", [D, DH], BF16, isOutput=False)
    wv = nc.declare_dram_parameter("wv", [D, DH], BF16, isOutput=False)
    wo = nc.declare_dram_parameter("wo", [DH, D], BF16, isOutput=False)
    tri = nc.declare_dram_parameter("tri", [128, 128], BF16, isOutput=False)
    outT = nc.declare_dram_parameter("outT", [D, S], BF16, isOutput=True)
    if debug:
        d_qT = nc.declare_dram_parameter("d_qT", [DH, S], BF16, isOutput=True)
        d_kT = nc.declare_dram_parameter("d_kT", [DH, S], BF16, isOutput=True)
        d_v = nc.declare_dram_parameter("d_v", [S, HL * (HD + 1)], BF16, isOutput=True)
        d_ctxT = nc.declare_dram_parameter("d_ctxT", [DH, S], BF16, isOutput=True)

    with tile.TileContext(nc) as tc, ExitStack() as ctx:
        const_pool = ctx.enter_context(tc.tile_pool(name="const", bufs=1))
        x_pool = ctx.enter_context(tc.tile_pool(name="x", bufs=1))
        w_pool = ctx.enter_context(tc.tile_pool(name="w", bufs=1))
        qk_pool = ctx.enter_context(tc.tile_pool(name="qk", bufs=1))
        v_pool = ctx.enter_context(tc.tile_pool(name="v", bufs=1))
        ctxT_pool = ctx.enter_context(tc.tile_pool(name="ctxT", bufs=1))
        e_pool = ctx.enter_context(tc.tile_pool(name="e", bufs=8))
        r_pool = ctx.enter_context(tc.tile_pool(name="r", bufs=1))
        o_pool = ctx.enter_context(tc.tile_pool(name="o", bufs=2))
        o3_pool = ctx.enter_context(tc.tile_pool(name="o3", bufs=1))
        ps_sp = ctx.enter_context(tc.tile_pool(name="ps_sp", bufs=2, space="PSUM"))
        ps_c = ctx.enter_context(tc.tile_pool(name="ps_c", bufs=1, space="PSUM"))
        ps_gen = ctx.enter_context(tc.tile_pool(name="ps_gen", bufs=2, space="PSUM"))

        # ---- constants ----
        trit = const_pool.tile([128, 128], BF16)
        nc.sync.dma_start(trit[:], tri[:])
        onesb = const_pool.tile([1, 64], BF16)
        nc.vector.memset(onesb[:], 1.0)

        # ---- inputs ----
        # Per-k 2D slices (multi-dim dram APs under-synchronize on HW).
        # Triggers are spread across engine queues so the front-critical
        # loads (wv + x superblock 0) issue in parallel.
        wvt = w_pool.tile([128, KT, DH], BF16, name="wvt")
        wqt = w_pool.tile([128, KT, DH], BF16, name="wqt")
        wkt = w_pool.tile([128, KT, DH], BF16, name="wkt")
        wot = w_pool.tile([128, DH // 128, D], BF16, name="wot")
        xs = [x_pool.tile([128, KT, 512], BF16, name=f"xs{_}") for _ in range(NB)]

        # Priority order on one queue: a DMA's descriptors go out before the
        # next trigger's, so the front-critical tensors finish first instead
        # of fair-sharing HBM bandwidth with the whole load set.
        def xsrc(s):
            return xT[:, 512 * s : 512 * (s + 1)].rearrange("(k p) c -> p k c", p=128)

        for k in range(KT):
            nc.sync.dma_start(wvt[:, k, :], wv[128 * k : 128 * (k + 1), :])
            nc.sync.dma_start(xs[0][:, k, :], xT[128 * k : 128 * (k + 1), 0:512])
        nc.sync.dma_start(wqt[:], wq.rearrange("(k p) c -> p k c", p=128)[:])
        nc.sync.dma_start(wkt[:], wk.rearrange("(k p) c -> p k c", p=128)[:])
        nc.sync.dma_start(xs[1][:], xsrc(1)[:])
        nc.sync.dma_start(xs[2][:], xsrc(2)[:])
        nc.sync.dma_start(xs[3][:], xsrc(3)[:])
        nc.sync.dma_start(wot[:], wo.rearrange("(k p) c -> p k c", p=128)[:])

        # ---- persistent activations ----
        qTt = [qk_pool.tile([128, S], BF16, name=f"qT{_}") for _ in range(NP)]
        kTt = [qk_pool.tile([128, S], BF16, name=f"kT{_}") for _ in range(NP)]
        vt = [v_pool.tile([128, HL * (HD + 1)], BF16, name=f"v{_}") for _ in range(ST)]
        ctxT = [ctxT_pool.tile([128, S], BF16, name=f"ctxT{_}") for _ in range(NP)]

        # ---- generation units ----
        # Each unit is one PSUM accumulation chain, split into two half-unit
        # closures (~4 matmuls each) for fine-grained interleaving with the
        # attention rounds.  Interleaving foreign matmuls inside an open
        # accumulation group is legal (per-bank has_written state).
        def emit_v(st):
            """V for k-tile st: [128, 8*(HD+1)] with a ones column per head."""
            state = {}
            xcol = xs[st // 4]

            def half_a():
                nc.vector.memset(
                    vt[st].rearrange("p (h c) -> p h c", c=HD + 1)[:, :, HD], 1.0
                )
                state["pv"] = ps_gen.tile([128, 512], F32, tag="pg", name=f"pv{st}")
                for k in range(KT // 2):
                    nc.tensor.matmul(
                        state["pv"][:],
                        xcol[:, k, 128 * (st % 4) : 128 * (st % 4 + 1)],
                        wvt[:, k, :],
                        start=(k == 0),
                        stop=False,
                    )

            def half_b():
                pv = state["pv"]
                for k in range(KT // 2, KT):
                    nc.tensor.matmul(
                        pv[:],
                        xcol[:, k, 128 * (st % 4) : 128 * (st % 4 + 1)],
                        wvt[:, k, :],
                        start=False,
                        stop=(k == KT - 1),
                    )
                nc.vector.tensor_copy(
                    vt[st].rearrange("p (h c) -> p h c", c=HD + 1)[:, :, 0:HD],
                    pv.rearrange("p (h c) -> p h c", c=HD)[:],
                )

            return [half_a, half_b]

        def emit_qk(wt, dst, m, n):
            """q^T or k^T for head-pair m, sequence superblock n.

            Emitted as four 2-matmul quarter-units for smooth fill pacing."""
            state = {}

            def quarter(q):
                def _q():
                    if q == 0:
                        state["ps"] = ps_gen.tile(
                            [128, 512], F32, tag="pg", name=f"pqk{m}_{n}"
                        )
                    ps = state["ps"]
                    for k in range(2 * q, 2 * q + 2):
                        nc.tensor.matmul(
                            ps[:],
                            wt[:, k, 128 * m : 128 * (m + 1)],
                            xs[n][:, k, :],
                            start=(k == 0),
                            stop=(k == KT - 1),
                        )
                    if q == 3:
                        nc.vector.tensor_copy(
                            dst[m][:, 512 * n : 512 * (n + 1)], ps[:]
                        )

                return _q

            return [quarter(q) for q in range(4)]

        def emit_op(m, n):
            """out^T rows [128m:128(m+1)], columns superblock n."""
            state = {}

            def half_a():
                state["ps"] = ps_gen.tile([128, 512], F32, tag="pg", name=f"pop{m}_{n}")
                for k in range(2):
                    nc.tensor.matmul(
                        state["ps"][:],
                        wot[:, k, 128 * m : 128 * (m + 1)],
                        ctxT[k][:, 512 * n : 512 * (n + 1)],
                        start=(k == 0),
                        stop=False,
                    )

            def half_b():
                ps = state["ps"]
                for k in range(2, DH // 128):
                    nc.tensor.matmul(
                        ps[:],
                        wot[:, k, 128 * m : 128 * (m + 1)],
                        ctxT[k][:, 512 * n : 512 * (n + 1)],
                        start=False,
                        stop=(k == DH // 128 - 1),
                    )
                ot = o_pool.tile([128, 512], BF16, tag="ot", name=f"ot{m}_{n}")
                nc.vector.tensor_copy(ot[:], ps[:])
                nc.sync.dma_start(
                    outT[128 * m : 128 * (m + 1), 512 * n : 512 * (n + 1)], ot[:]
                )

            return [half_a, half_b]

        # Final column-superblock out-proj, split so only a rank-128 update
        # plus an add remains after the last head finishes.
        o3_tiles = {}

        def emit_op3_partial(m):
            def unit():
                ps = ps_gen.tile([128, 512], F32, tag="pg", name=f"pop3a{m}")
                for k in range(3):
                    nc.tensor.matmul(
                        ps[:],
                        wot[:, k, 128 * m : 128 * (m + 1)],
                        ctxT[k][:, 1536:2048],
                        start=(k == 0),
                        stop=(k == 2),
                    )
                t = o3_pool.tile([128, 512], F32, tag=f"o3_{m}", name=f"o3_{m}")
                nc.vector.tensor_copy(t[:], ps[:])
                o3_tiles[m] = t

            return [unit]

        def emit_op3_final(m):
            def unit():
                ps = ps_gen.tile([128, 512], F32, tag="pg", name=f"pop3b{m}")
                nc.tensor.matmul(
                    ps[:],
                    wot[:, 3, 128 * m : 128 * (m + 1)],
                    ctxT[3][:, 1536:2048],
                    start=True,
                    stop=True,
                )
                ot = o_pool.tile([128, 512], BF16, tag="ot", name=f"ot3_{m}")
                nc.vector.tensor_tensor(
                    ot[:], o3_tiles[m][:], ps[:], mybir.AluOpType.add
                )
                nc.sync.dma_start(outT[128 * m : 128 * (m + 1), 1536:2048], ot[:])

            return [unit]

        # ---- attention ----
        pending = []  # deferred normalization closures

        def make_norm(p, I, X, cps):
            def _norm():
                cun = r_pool.tile([65, 512], F32, tag=f"cun{X}", name="cun")
                nc.vector.tensor_copy(cun[:], cps[:])
                # den row must move to partition 0: reciprocal_approx_fast is
                # custom DVE ucode and corrupts SBUF at a nonzero base
                # partition (HW-only; CoreSim doesn't model it).
                den0 = r_pool.tile([1, 512], F32, tag="den0", name="den0")
                nc.sync.dma_start(den0[0:1, :], cun[64:65, :])
                rec = r_pool.tile([1, 512], F32, tag="rec", name="rec")
                nc.vector.reciprocal_approx_fast(rec[0:1, :], den0[0:1, :])
                recb = r_pool.tile([1, 512], BF16, tag="recb", name="recb")
                nc.vector.tensor_copy(recb[0:1, :], rec[0:1, :])
                bc = ps_c.tile([65, 512], F32, tag=f"c{X}", name="bc")
                nc.tensor.matmul(
                    bc[0:64, :], onesb[0:1, 0:64], recb[0:1, :],
                    start=True, stop=True,
                )
                dst = ctxT[p][64 * X : 64 * X + 64, 512 * I : 512 * (I + 1)]
                if X == 0:
                    nc.vector.tensor_tensor(dst, cun[0:64, :], bc[0:64, :], Mult)
                else:
                    nrm = r_pool.tile([64, 512], BF16, tag="nrm", name="nrm")
                    nc.vector.tensor_tensor(nrm[:], cun[0:64, :], bc[0:64, :], Mult)
                    nc.sync.dma_start(dst, nrm[:])

            return _norm

        def attn_pair(p, I, fillers, budget, urgent=None):
            """All k-rounds for head pair p, q-superblock I.

            fillers: deque of half-unit closures; budget[0] accumulates the
            fill pacing fraction per round.  urgent: closures popped one per
            round ahead of the budgeted fillers (deadline-critical units)."""
            nj = 4 * I + 4
            cps = [
                ps_c.tile([65, 512], F32, tag=f"c{X}", name=f"cps{X}")
                for X in range(2)
            ]
            for j in range(nj):
                diag = (j // 4 == I)
                lo = 128 * (j - 4 * I) if diag else 0
                sp = ps_sp.tile([128, 1024], F32, tag="sp", name="sp")
                if lo > 0:
                    # B-head's masked hole would be uninitialized PSUM under
                    # the single merged exp below.
                    nc.vector.memset(sp[:, 512 : 512 + lo], 0.0)
                for X in range(2):
                    nc.tensor.matmul(
                        sp[:, 512 * X + lo : 512 * (X + 1)],
                        kTt[p][64 * X : 64 * X + 64, 128 * j : 128 * (j + 1)],
                        qTt[p][64 * X : 64 * X + 64, 512 * I + lo : 512 * (I + 1)],
                        start=True,
                        stop=True,
                    )
                e = e_pool.tile([128, 1024], BF16, tag="e", name="e")
                nc.scalar.activation(
                    e[:, lo:1024], sp[:, lo:1024], Exp, scale=float(SCALE)
                )
                if diag:
                    for X in range(2):
                        nc.vector.tensor_tensor(
                            e[:, 512 * X + lo : 512 * X + lo + 128],
                            e[:, 512 * X + lo : 512 * X + lo + 128],
                            trit[:],
                            Mult,
                        )
                while pending:
                    pending.pop(0)()
                if urgent:
                    urgent.pop(0)()
                budget[0] += budget[1]
                while budget[0] >= 1.0 and fillers:
                    fillers.pop(0)()
                    budget[0] -= 1.0
                for X in range(2):
                    nc.tensor.matmul(
                        cps[X][:, lo:512],
                        vt[j][:, (HD + 1) * (2 * p + X) : (HD + 1) * (2 * p + X + 1)],
                        e[:, 512 * X + lo : 512 * (X + 1)],
                        start=(j == 0),
                        stop=(j == nj - 1),
                        skip_group_check=True,
                    )
            for X in range(2):
                pending.append(make_norm(p, I, X, cps[X]))

        # ---- emission schedule ----
        def run_all(units):
            for u in units:
                for half in u:
                    half()

        def flat(units):
            return [half for u in units for half in u]

        # upfront: V k-tiles 0-3 and q/k for pair 0, superblock 0
        run_all([emit_v(st) for st in range(4)])
        run_all([emit_qk(wqt, qTt, 0, 0), emit_qk(wkt, kTt, 0, 0)])

        phase_fillers = [
            # during sb0: remaining sb0 q/k, V 4-7, all of sb1 q/k
            flat(
                [emit_qk(wqt, qTt, m, 0) for m in range(1, NP)]
                + [emit_qk(wkt, kTt, m, 0) for m in range(1, NP)]
                + [emit_v(st) for st in range(4, 8)]
                + [emit_qk(wqt, qTt, m, 1) for m in range(NP)]
                + [emit_qk(wkt, kTt, m, 1) for m in range(NP)]
            ),
            # during sb1: sb2 q/k
            flat(
                [emit_qk(wqt, qTt, m, 2) for m in range(NP)]
                + [emit_qk(wkt, kTt, m, 2) for m in range(NP)]
            ),
            # during sb2: sb3 q/k (V 8-11 in the urgent lane)
            flat(
                [emit_qk(wqt, qTt, m, 3) for m in range(NP)]
                + [emit_qk(wkt, kTt, m, 3) for m in range(NP)]
            ),
            # during sb3: all deferrable out-proj columns (V 12-15 in the
            # urgent lane: needed by round 12 = pair 0's j=12)
            flat(
                [emit_op(m, 0) for m in range(D // 128)]
                + [emit_op(m, 1) for m in range(D // 128)]
                + [emit_op(m, 2) for m in range(D // 128)]
            ),
        ]
        # urgent lanes, popped one per round ahead of budgeted fillers
        # (deadline-critical V generation for the next superblock's k-tiles)
        urgent_lanes = {
            (2, 0): flat([emit_v(st) for st in range(8, 12)]),
            (3, 0): flat([emit_v(st) for st in range(12, 16)]),
        }

        for I in range(NB):
            fillers = phase_fillers[I]
            rounds = NP * (4 * I + 4)
            budget = [0.999, len(fillers) / rounds]
            for p in range(NP):
                urgent = urgent_lanes.get((I, p), [])
                attn_pair(p, I, fillers, budget, urgent)
                while urgent:
                    urgent.pop(0)()
            while fillers:
                fillers.pop(0)()
        while pending:
            pending.pop(0)()
        run_all([emit_op(m, 3) for m in range(D // 128)])

        if debug:
            for p in range(NP):
                nc.sync.dma_start(d_qT[128 * p : 128 * (p + 1), :], qTt[p][:])
                nc.sync.dma_start(d_kT[128 * p : 128 * (p + 1), :], kTt[p][:])
                nc.sync.dma_start(d_ctxT[128 * p : 128 * (p + 1), :], ctxT[p][:])
            for st in range(ST):
                nc.sync.dma_start(d_v[128 * st : 128 * (st + 1), :], vt[st][:])

    nc.compile()
    return nc


_NC_CACHE = None


def kernel(x, Wq, Wk, Wv, Wo, bo):
    global _NC_CACHE
    if _NC_CACHE is None:
        _NC_CACHE = _build_nc()
    nc = _NC_CACHE

    bf = ml_dtypes.bfloat16
    tri = np.triu(np.ones((128, 128), dtype=np.float32)).astype(bf)
    in_maps = []
    for c in range(NC):
        b, hg = c // 2, c % 2
        cols = slice(DH * hg, DH * (hg + 1))
        in_maps.append(
            {
                "xT": np.ascontiguousarray(np.asarray(x)[b].T).astype(bf),
                "wq": np.asarray(Wq)[:, cols].astype(bf),
                "wk": np.asarray(Wk)[:, cols].astype(bf),
                "wv": np.asarray(Wv)[:, cols].astype(bf),
                "wo": np.asarray(Wo)[cols, :].astype(bf),
                "tri": tri,
            }
        )
    res = run_bass_kernel_spmd(nc, in_maps, core_ids=list(range(NC)))
    out = np.empty((B, S, D), dtype=np.float32)
    bo32 = np.asarray(bo, dtype=np.float32)
    for b in range(B):
        acc = res.results[2 * b]["outT"].astype(np.float32) + res.results[2 * b + 1][
            "outT"
        ].astype(np.float32)
        out[b] = acc.T + bo32
    return out
